# Initial kernel scaffold
#
"""Trainium2 Bass kernel for an 8-head attention layer (B=4, T=2048, K=512, H=8).

Sharding: DP=4 over batch x TP=2 over heads across 8 NeuronCores.
Core c handles batch c//2 with heads [4*(c%2), 4*(c%2)+4).

Algebraic folding (host side):
  scores = (x Wq)(x Wk)^T / sqrt(K) = x M x^T   with M_h = Wq_h Wk_h^T / sqrt(K)
  out    = sum_h A_h (x Wv_h) Wu_h = sum_h A_h (x G_h)  with G_h = Wv_h Wu_h
This removes the K projection and the whole unify matmul stage (~17% of
tensor-engine flops). The per-head output sum accumulates on the vector
engine; the core pair ReduceScatters the final partials.

The scores matmul runs in fp8(e4m3) DoubleRow mode (2x PE throughput):
x is quantized host-side (scale 32), q' = x M is cast f32->fp8 (scale 1024)
on the scalar engine straight out of PSUM; the exp activation folds the
1/32768 descale. Everything else is bf16 with f32 PSUM accumulation.
Measured end-to-end relative error ~1.3e-2 (gate 2e-2); bf16-only is ~2.5e-3.
"""

import numpy as np
import ml_dtypes

# Problem constants (hardcoded; kernel.py must be self-contained).
B, T, K, H = 4, 2048, 512, 8
NCORES = 8
HL = H // 2        # heads per core (TP=2)
P = 128
KC = K // P        # feature chunks = 4
TC = T // P        # token/key chunks of 128 = 16
QB = 4             # query blocks of 512
NQC = 4            # 128-query subchunks per block
SX = 32.0          # x fp8 scale
SQ = 1024.0        # q' fp8 scale
ESC = 1.0 / (SX * SQ)
REPLICA_GROUPS = [[0, 1], [2, 3], [4, 5], [6, 7]]

_NC_CACHE = {}


def _build_nc():
    import concourse.mybir as mybir
    import concourse.tile as tile
    from concourse import bacc

    f32 = mybir.dt.float32
    bf16 = mybir.dt.bfloat16
    fp8 = mybir.dt.float8e4
    Exp = mybir.ActivationFunctionType.Exp
    Copy = mybir.ActivationFunctionType.Copy
    DR = mybir.MatmulPerfMode.DoubleRow
    MUL = mybir.AluOpType.mult
    ADD = mybir.AluOpType.add

    nc = bacc.Bacc("TRN2", target_bir_lowering=False, debug=False,
                   num_devices=NCORES)

    xT_d = nc.dram_tensor("xT", [K, T], bf16, kind="ExternalInput")
    x8_d = nc.dram_tensor("x8", [K, T], fp8, kind="ExternalInput")
    wm_d = nc.dram_tensor("wm", [K, HL * K], bf16, kind="ExternalInput")
    wg_d = nc.dram_tensor("wg", [K, HL * K], bf16, kind="ExternalInput")
    bias_d = nc.dram_tensor("bias_bc", [P, K], f32, kind="ExternalInput")
    out_d = nc.dram_tensor("out", [T // 2, K], bf16, kind="ExternalOutput")
    # DRAM bounce buffers for the per-query-block ReduceScatter.
    # Partials are exchanged in bf16 (the pair-sum happens in the collective);
    # the host casts back to f32.
    cc_ins = [nc.dram_tensor(f"cc_in{i}", [512, K], bf16, kind="Internal")
              for i in range(QB)]
    cc_outs = [nc.dram_tensor(f"cc_out{i}", [256, K], bf16, kind="Internal")
               for i in range(QB)]

    with tile.TileContext(nc) as tc:
        with (
            tc.tile_pool(name="const", bufs=1) as constp,
            tc.tile_pool(name="big", bufs=1) as bigp,
            tc.tile_pool(name="wpool", bufs=2) as wpool,
            tc.tile_pool(name="qkv", bufs=1) as qkvp,
            tc.tile_pool(name="attn", bufs=4) as attnp,
            tc.tile_pool(name="outp", bufs=2) as outp,
            tc.tile_pool(name="ps_mm", bufs=2, space="PSUM") as ps_mm,
            tc.tile_pool(name="ps_o", bufs=4, space="PSUM") as ps_o,
            tc.tile_pool(name="ps_sum", bufs=2, space="PSUM") as ps_sum,
        ):
            ones = constp.tile([P, 1], bf16)
            nc.vector.memset(ones[:], 1.0)
            bias_sb = constp.tile([P, K], f32)
            nc.gpsimd.dma_start(bias_sb[:], bias_d[:, :])

            # x^T resident in SBUF: bf16 for projections, fp8 for scores.
            # Four separate 512-column tiles so the first projection matmuls
            # only depend on the first block's DMA; x8 is only needed in the
            # attention phase so it loads after head 0's weights.
            xT_sbs = []
            for tb in range(QB):
                tsl = slice(tb * 512, (tb + 1) * 512)
                t = bigp.tile([P, KC, 512], bf16, name=f"xT{tb}",
                              tag=f"xT{tb}")
                nc.sync.dma_start(
                    t[:], xT_d.ap()[:, tsl].rearrange("(c p) t -> p c t", p=P))
                xT_sbs.append(t)
            x8_sb = bigp.tile([P, KC, T], fp8)

            # ---- projection phase: q'8 = fp8(x M_h), v' = bf16(x G_h) ----
            q8s, vs = [], []
            for h in range(HL):
                col = slice(h * K, (h + 1) * K)
                # Head 0's weights go on the gpsimd DMA queue so they land in
                # parallel with the xT blocks on the sync queue (M split by
                # output chunk so the very first matmul group only waits for
                # a 128KB slice). Later heads' weights and x8 are triggered
                # from the scalar queue, after compute has started, to keep
                # the DMA rings free for the critical startup loads.
                m_sb = wpool.tile([P, KC, K], bf16, tag="wm")
                if h == 0:
                    for dc in range(KC):
                        dsl = slice(h * K + dc * P, h * K + (dc + 1) * P)
                        nc.gpsimd.dma_start(
                            m_sb[:, :, dc * P:(dc + 1) * P],
                            wm_d.ap()[:, dsl].rearrange(
                                "(c p) d -> p c d", p=P))
                else:
                    nc.scalar.dma_start(
                        m_sb[:],
                        wm_d.ap()[:, col].rearrange("(c p) d -> p c d", p=P))
                g_sb = wpool.tile([P, KC, K], bf16, tag="wg")
                weng = nc.gpsimd if h == 0 else nc.scalar
                weng.dma_start(
                    g_sb[:],
                    wg_d.ap()[:, col].rearrange("(c p) d -> p c d", p=P))

                q8_t = qkvp.tile([P, KC, T], fp8, name=f"q8_{h}", tag=f"q8_{h}")
                v_t = qkvp.tile([P, TC, K], bf16, name=f"v_{h}", tag=f"v_{h}")
                q8s.append(q8_t)
                vs.append(v_t)

                for tb in range(QB):
                    for dc in range(KC):
                        ps = ps_mm.tile([P, 512], f32, tag="mm")
                        for kc in range(KC):
                            nc.tensor.matmul(
                                ps[:],
                                m_sb[:, kc, dc * P:(dc + 1) * P],
                                xT_sbs[tb][:, kc, :],
                                start=(kc == 0), stop=(kc == KC - 1))
                        nc.scalar.activation(
                            q8_t[:, dc, tb * 512:(tb + 1) * 512], ps[:],
                            Copy, scale=SQ)
                        if h == 0 and tb == 0 and dc == 0:
                            nc.scalar.dma_start(
                                x8_sb[:],
                                x8_d.ap().rearrange("(c p) t -> p c t", p=P))
                for t16 in range(TC):
                    tb, tw = t16 // 4, t16 % 4
                    ps = ps_mm.tile([P, 512], f32, tag="mm")
                    for kc in range(KC):
                        nc.tensor.matmul(
                            ps[:],
                            xT_sbs[tb][:, kc, tw * P:(tw + 1) * P],
                            g_sb[:, kc, :],
                            start=(kc == 0), stop=(kc == KC - 1))
                    nc.vector.tensor_copy(v_t[:, t16, :], ps[:])

            # ---- attention phase ----
            out_accs = {}

            def flush(st):
                # Finalize head h of query block qb: per-query sums
                # (tiny N=1 matmuls reduce the partition dim), reciprocal,
                # then out_acc (+)= opsum * rinv on the vector engine.
                h, qb, opsums, sum_acc = st
                sps = ps_sum.tile([P, NQC], f32, tag="sum")
                for qc in range(NQC):
                    nc.tensor.matmul(
                        sps[:, qc:qc + 1],
                        sum_acc[:, qc * P:(qc + 1) * P],
                        ones[:, :1], start=True, stop=True)
                rv = attnp.tile([P, NQC], f32, tag="rinv", bufs=2)
                nc.vector.reciprocal(rv[:], sps[:])
                oacc = out_accs[qb]
                last = h == HL - 1
                if last:
                    oacc8 = outp.tile([P, NQC, K], bf16, tag="oacc8",
                                      name=f"oacc8_{qb}")
                for qc in range(NQC):
                    in1 = bias_sb[:] if h == 0 else oacc[:, qc, :]
                    dst = oacc8[:, qc, :] if last else oacc[:, qc, :]
                    nc.vector.scalar_tensor_tensor(
                        dst, opsums[qc][:], rv[:, qc:qc + 1],
                        in1, MUL, ADD)
                    if last:
                        nc.sync.dma_start(
                            cc_ins[qb][qc * P:(qc + 1) * P, :],
                            oacc8[:, qc, :])
                if last:
                    # query block complete: ReduceScatter the 512-token
                    # chunk across the core pair, then DMA to the output.
                    nc.gpsimd.collective_compute(
                        "ReduceScatter",
                        mybir.AluOpType.add,
                        replica_groups=REPLICA_GROUPS,
                        ins=[cc_ins[qb][:, :]],
                        outs=[cc_outs[qb][:, :]],
                    )
                    nc.sync.dma_start(
                        out_d[qb * 256:(qb + 1) * 256, :], cc_outs[qb][:, :])

            prev = None
            for qb in range(QB):
                qsl = slice(qb * 512, (qb + 1) * 512)
                out_accs[qb] = outp.tile([P, NQC, K], f32, tag="oacc",
                                         name=f"oacc{qb}")
                for h in range(HL):
                    opsums = [ps_o.tile([P, 512], f32, tag="o",
                                        name=f"o{qb}_{h}_{qc}")
                              for qc in range(NQC)]
                    sum_acc = attnp.tile([P, 512], bf16, tag="sacc", bufs=2)

                    def emit_av(pend_item):
                        e_p, k_p = pend_item
                        for qc in range(NQC):
                            nc.tensor.matmul(
                                opsums[qc][:],
                                e_p[:, qc * P:(qc + 1) * P],
                                vs[h][:, k_p, :],
                                start=(k_p == 0), stop=(k_p == TC - 1))

                    pend = []
                    for kc16 in range(TC):
                        ps = ps_mm.tile([P, 512], f32, tag="mm")
                        for j in range(2):
                            nc.tensor.matmul(
                                ps[:],
                                x8_sb[:, 2 * j:2 * j + 2,
                                      kc16 * P:(kc16 + 1) * P],
                                q8s[h][:, 2 * j:2 * j + 2, qsl],
                                start=(j == 0), stop=(j == 1),
                                perf_mode=DR)
                        if kc16 == 0 and prev is not None:
                            flush(prev)
                        if len(pend) >= 2:
                            emit_av(pend.pop(0))
                        e_ch = attnp.tile([P, 512], bf16, tag="e", bufs=4)
                        nc.scalar.activation(e_ch[:], ps[:], Exp, bias=0.0,
                                             scale=ESC)
                        if kc16 == 0:
                            nc.vector.tensor_copy(sum_acc[:], e_ch[:])
                        else:
                            nc.vector.tensor_add(sum_acc[:], sum_acc[:],
                                                 e_ch[:])
                        pend.append((e_ch, kc16))
                    for item in pend:
                        emit_av(item)
                    prev = (h, qb, opsums, sum_acc)
            flush(prev)

    nc.compile()
    return nc


def _get_nc():
    if "nc" not in _NC_CACHE:
        _NC_CACHE["nc"] = _build_nc()
    return _NC_CACHE["nc"]


def _make_in_maps(x, Wq, Wk, Wv, Wu, bu):
    f32 = np.float32
    bf16 = ml_dtypes.bfloat16
    fp8 = ml_dtypes.float8_e4m3
    inv2 = 1.0 / np.sqrt(K)
    Wq32 = np.asarray(Wq, f32)
    Wk32 = np.asarray(Wk, f32)
    Wv32 = np.asarray(Wv, f32)
    Wu32 = np.asarray(Wu, f32)
    M = np.empty((H, K, K), f32)
    G = np.empty((H, K, K), f32)
    for h in range(H):
        hs = slice(h * K, (h + 1) * K)
        M[h] = (Wq32[:, hs] @ Wk32[:, hs].T) * inv2
        G[h] = Wv32[:, hs] @ Wu32[hs, :]
    bias_bc = np.ascontiguousarray(
        np.broadcast_to((np.asarray(bu, f32) * 0.5)[None, :], (P, K)))
    in_maps = []
    for c in range(NCORES):
        b, r = c // 2, c % 2
        xb = np.asarray(x[b], f32)
        xT = np.ascontiguousarray(xb.T)
        heads = range(r * HL, r * HL + HL)
        wm = np.concatenate([M[h] for h in heads], axis=1)
        wg = np.concatenate([G[h] for h in heads], axis=1)
        in_maps.append({
            "xT": xT.astype(bf16),
            "x8": np.clip(xT * SX, -240.0, 240.0).astype(fp8),
            "wm": wm.astype(bf16),
            "wg": wg.astype(bf16),
            "bias_bc": bias_bc,
        })
    return in_maps


def _assemble(results):
    out = np.empty((B, T, K), np.float32)
    for c in range(NCORES):
        b, r = c // 2, c % 2
        o = np.asarray(results[c]["out"], np.float32)
        for qb in range(QB):  # 4 chunks of 256 tokens
            t0 = qb * 512 + r * 256
            out[b, t0:t0 + 256] = o[qb * 256:(qb + 1) * 256]
    return out


def run_on_hw(x, Wq, Wk, Wv, Wu, bu, trace=False):
    from concourse.bass_utils import run_bass_kernel_spmd
    nc = _get_nc()
    in_maps = _make_in_maps(x, Wq, Wk, Wv, Wu, bu)
    res = run_bass_kernel_spmd(nc, in_maps, core_ids=list(range(NCORES)),
                               trace=trace)
    return _assemble(res.results), res


def kernel(x, Wq, Wk, Wv, Wu, bu):
    out, _ = run_on_hw(x, Wq, Wk, Wv, Wu, bu, trace=False)
    return out



# revision 2
# speedup vs baseline: 4.0124x; 4.0124x over previous
"""Trainium2 Bass kernel for an 8-head attention layer (B=4, T=2048, K=512, H=8).

Sharding: DP=4 over batch x TP=2 over heads across 8 NeuronCores.
Core c handles batch c//2 with heads [4*(c%2), 4*(c%2)+4).

Algebraic folding (host side):
  scores = (x Wq)(x Wk)^T / sqrt(K) = x M x^T   with M_h = Wq_h Wk_h^T / sqrt(K)
  out    = sum_h A_h (x Wv_h) Wu_h = sum_h A_h (x G_h)  with G_h = Wv_h Wu_h
This removes the K projection and the whole unify matmul stage (~17% of
tensor-engine flops). The per-head output sum accumulates on the vector
engine; the core pair ReduceScatters the final partials.

The scores matmul runs in fp8(e4m3) DoubleRow mode (2x PE throughput):
x is quantized host-side (scale 32), q' = x M is cast f32->fp8 (scale 1024)
on the scalar engine straight out of PSUM; the exp activation folds the
1/32768 descale. Everything else is bf16 with f32 PSUM accumulation.
Measured end-to-end relative error ~1.3e-2 (gate 2e-2); bf16-only is ~2.5e-3.
"""

import numpy as np
import ml_dtypes

# Problem constants (hardcoded; kernel.py must be self-contained).
B, T, K, H = 4, 2048, 512, 8
NCORES = 8
HL = H // 2        # heads per core (TP=2)
P = 128
KC = K // P        # feature chunks = 4
TC = T // P        # token/key chunks of 128 = 16
QB = 4             # query blocks of 512
NQC = 4            # 128-query subchunks per block
SX = 32.0          # x fp8 scale
SQ = 1024.0        # q' fp8 scale
ESC = 1.0 / (SX * SQ)
REPLICA_GROUPS = [[0, 1], [2, 3], [4, 5], [6, 7]]

_NC_CACHE = {}


def _build_nc():
    import concourse.mybir as mybir
    import concourse.tile as tile
    from concourse import bacc

    f32 = mybir.dt.float32
    bf16 = mybir.dt.bfloat16
    fp8 = mybir.dt.float8e4
    Exp = mybir.ActivationFunctionType.Exp
    Copy = mybir.ActivationFunctionType.Copy
    DR = mybir.MatmulPerfMode.DoubleRow
    MUL = mybir.AluOpType.mult
    ADD = mybir.AluOpType.add

    nc = bacc.Bacc("TRN2", target_bir_lowering=False, debug=False,
                   num_devices=NCORES)

    xT_d = nc.dram_tensor("xT", [K, T], bf16, kind="ExternalInput")
    x8_d = nc.dram_tensor("x8", [K, T], fp8, kind="ExternalInput")
    wm_d = nc.dram_tensor("wm", [K, HL * K], bf16, kind="ExternalInput")
    wg_d = nc.dram_tensor("wg", [K, HL * K], bf16, kind="ExternalInput")
    bias_d = nc.dram_tensor("bias_bc", [P, K], f32, kind="ExternalInput")
    out_d = nc.dram_tensor("out", [T // 2, K], bf16, kind="ExternalOutput")
    # DRAM bounce buffers for the per-query-block ReduceScatter.
    # Partials are exchanged in bf16 (the pair-sum happens in the collective);
    # the host casts back to f32.
    cc_ins = [nc.dram_tensor(f"cc_in{i}", [512, K], bf16, kind="Internal")
              for i in range(QB)]
    cc_outs = [nc.dram_tensor(f"cc_out{i}", [256, K], bf16, kind="Internal")
               for i in range(QB)]

    with tile.TileContext(nc) as tc:
        with (
            tc.tile_pool(name="const", bufs=1) as constp,
            tc.tile_pool(name="big", bufs=1) as bigp,
            tc.tile_pool(name="wpool", bufs=2) as wpool,
            tc.tile_pool(name="qkv", bufs=1) as qkvp,
            tc.tile_pool(name="attn", bufs=4) as attnp,
            tc.tile_pool(name="outp", bufs=2) as outp,
            tc.tile_pool(name="ps_mm", bufs=2, space="PSUM") as ps_mm,
            tc.tile_pool(name="ps_o", bufs=4, space="PSUM") as ps_o,
            tc.tile_pool(name="ps_sum", bufs=2, space="PSUM") as ps_sum,
        ):
            ones = constp.tile([P, 1], bf16)
            nc.vector.memset(ones[:], 1.0)
            bias_sb = constp.tile([P, K], f32)
            nc.gpsimd.dma_start(bias_sb[:], bias_d[:, :])

            # x^T resident in SBUF: bf16 for projections, fp8 for scores.
            # Four separate 512-column tiles so the first projection matmuls
            # only depend on the first block's DMA; x8 is only needed in the
            # attention phase so it loads after head 0's weights.
            xT_sbs = []
            for tb in range(QB):
                tsl = slice(tb * 512, (tb + 1) * 512)
                t = bigp.tile([P, KC, 512], bf16, name=f"xT{tb}",
                              tag=f"xT{tb}")
                nc.sync.dma_start(
                    t[:], xT_d.ap()[:, tsl].rearrange("(c p) t -> p c t", p=P))
                xT_sbs.append(t)
            x8_sb = bigp.tile([P, KC, T], fp8)

            # ---- projection phase: q'8 = fp8(x M_h), v' = bf16(x G_h) ----
            q8s, vs = [], []
            for h in range(HL):
                col = slice(h * K, (h + 1) * K)
                # Head 0's weights go on the gpsimd DMA queue so they land in
                # parallel with the xT blocks on the sync queue (M split by
                # output chunk so the very first matmul group only waits for
                # a 128KB slice). Later heads' weights and x8 are triggered
                # from the scalar queue, after compute has started, to keep
                # the DMA rings free for the critical startup loads.
                m_sb = wpool.tile([P, KC, K], bf16, tag="wm")
                if h == 0:
                    for dc in range(KC):
                        dsl = slice(h * K + dc * P, h * K + (dc + 1) * P)
                        nc.gpsimd.dma_start(
                            m_sb[:, :, dc * P:(dc + 1) * P],
                            wm_d.ap()[:, dsl].rearrange(
                                "(c p) d -> p c d", p=P))
                else:
                    nc.scalar.dma_start(
                        m_sb[:],
                        wm_d.ap()[:, col].rearrange("(c p) d -> p c d", p=P))
                g_sb = wpool.tile([P, KC, K], bf16, tag="wg")
                weng = nc.gpsimd if h == 0 else nc.scalar
                weng.dma_start(
                    g_sb[:],
                    wg_d.ap()[:, col].rearrange("(c p) d -> p c d", p=P))

                q8_t = qkvp.tile([P, KC, T], fp8, name=f"q8_{h}", tag=f"q8_{h}")
                v_t = qkvp.tile([P, TC, K], bf16, name=f"v_{h}", tag=f"v_{h}")
                q8s.append(q8_t)
                vs.append(v_t)

                for tb in range(QB):
                    for dc in range(KC):
                        ps = ps_mm.tile([P, 512], f32, tag="mm")
                        for kc in range(KC):
                            nc.tensor.matmul(
                                ps[:],
                                m_sb[:, kc, dc * P:(dc + 1) * P],
                                xT_sbs[tb][:, kc, :],
                                start=(kc == 0), stop=(kc == KC - 1))
                        nc.scalar.activation(
                            q8_t[:, dc, tb * 512:(tb + 1) * 512], ps[:],
                            Copy, scale=SQ)
                        if h == 0 and tb == 0 and dc == 0:
                            nc.scalar.dma_start(
                                x8_sb[:],
                                x8_d.ap().rearrange("(c p) t -> p c t", p=P))
                for t16 in range(TC):
                    tb, tw = t16 // 4, t16 % 4
                    ps = ps_mm.tile([P, 512], f32, tag="mm")
                    for kc in range(KC):
                        nc.tensor.matmul(
                            ps[:],
                            xT_sbs[tb][:, kc, tw * P:(tw + 1) * P],
                            g_sb[:, kc, :],
                            start=(kc == 0), stop=(kc == KC - 1))
                    nc.vector.tensor_copy(v_t[:, t16, :], ps[:])

            # ---- attention phase ----
            out_accs = {}

            def flush(st):
                # Finalize head h of query block qb: per-query sums
                # (tiny N=1 matmuls reduce the partition dim), reciprocal,
                # then out_acc (+)= opsum * rinv on the vector engine.
                h, qb, opsums, sum_acc = st
                sps = ps_sum.tile([P, NQC], f32, tag="sum")
                for qc in range(NQC):
                    nc.tensor.matmul(
                        sps[:, qc:qc + 1],
                        sum_acc[:, qc * P:(qc + 1) * P],
                        ones[:, :1], start=True, stop=True)
                rv = attnp.tile([P, NQC], f32, tag="rinv", bufs=2)
                nc.vector.reciprocal(rv[:], sps[:])
                oacc = out_accs[qb]
                last = h == HL - 1
                if last:
                    oacc8 = outp.tile([P, NQC, K], bf16, tag="oacc8",
                                      name=f"oacc8_{qb}")
                for qc in range(NQC):
                    in1 = bias_sb[:] if h == 0 else oacc[:, qc, :]
                    dst = oacc8[:, qc, :] if last else oacc[:, qc, :]
                    nc.vector.scalar_tensor_tensor(
                        dst, opsums[qc][:], rv[:, qc:qc + 1],
                        in1, MUL, ADD)
                    if last:
                        nc.sync.dma_start(
                            cc_ins[qb][qc * P:(qc + 1) * P, :],
                            oacc8[:, qc, :])
                if last:
                    # query block complete: ReduceScatter the 512-token
                    # chunk across the core pair, then DMA to the output.
                    nc.gpsimd.collective_compute(
                        "ReduceScatter",
                        mybir.AluOpType.add,
                        replica_groups=REPLICA_GROUPS,
                        ins=[cc_ins[qb][:, :]],
                        outs=[cc_outs[qb][:, :]],
                    )
                    nc.sync.dma_start(
                        out_d[qb * 256:(qb + 1) * 256, :], cc_outs[qb][:, :])

            prev = None
            for qb in range(QB):
                qsl = slice(qb * 512, (qb + 1) * 512)
                out_accs[qb] = outp.tile([P, NQC, K], f32, tag="oacc",
                                         name=f"oacc{qb}")
                for h in range(HL):
                    opsums = [ps_o.tile([P, 512], f32, tag="o",
                                        name=f"o{qb}_{h}_{qc}")
                              for qc in range(NQC)]
                    sum_acc = attnp.tile([P, 512], bf16, tag="sacc", bufs=2)

                    def emit_av(pend_item):
                        e_p, k_p = pend_item
                        for qc in range(NQC):
                            nc.tensor.matmul(
                                opsums[qc][:],
                                e_p[:, qc * P:(qc + 1) * P],
                                vs[h][:, k_p, :],
                                start=(k_p == 0), stop=(k_p == TC - 1))

                    pend = []
                    for kc16 in range(TC):
                        ps = ps_mm.tile([P, 512], f32, tag="mm")
                        for j in range(2):
                            nc.tensor.matmul(
                                ps[:],
                                x8_sb[:, 2 * j:2 * j + 2,
                                      kc16 * P:(kc16 + 1) * P],
                                q8s[h][:, 2 * j:2 * j + 2, qsl],
                                start=(j == 0), stop=(j == 1),
                                perf_mode=DR)
                        if kc16 == 0 and prev is not None:
                            flush(prev)
                        if len(pend) >= 2:
                            emit_av(pend.pop(0))
                        e_ch = attnp.tile([P, 512], bf16, tag="e", bufs=4)
                        nc.scalar.activation(e_ch[:], ps[:], Exp, bias=0.0,
                                             scale=ESC)
                        if kc16 == 0:
                            nc.vector.tensor_copy(sum_acc[:], e_ch[:])
                        else:
                            nc.vector.tensor_add(sum_acc[:], sum_acc[:],
                                                 e_ch[:])
                        pend.append((e_ch, kc16))
                    for item in pend:
                        emit_av(item)
                    prev = (h, qb, opsums, sum_acc)
            flush(prev)

    nc.compile()
    return nc


def _get_nc():
    if "nc" not in _NC_CACHE:
        _NC_CACHE["nc"] = _build_nc()
    return _NC_CACHE["nc"]


def _make_in_maps(x, Wq, Wk, Wv, Wu, bu):
    f32 = np.float32
    bf16 = ml_dtypes.bfloat16
    fp8 = ml_dtypes.float8_e4m3
    inv2 = 1.0 / np.sqrt(K)
    Wq32 = np.asarray(Wq, f32)
    Wk32 = np.asarray(Wk, f32)
    Wv32 = np.asarray(Wv, f32)
    Wu32 = np.asarray(Wu, f32)
    M = np.empty((H, K, K), f32)
    G = np.empty((H, K, K), f32)
    for h in range(H):
        hs = slice(h * K, (h + 1) * K)
        M[h] = (Wq32[:, hs] @ Wk32[:, hs].T) * inv2
        G[h] = Wv32[:, hs] @ Wu32[hs, :]
    bias_bc = np.ascontiguousarray(
        np.broadcast_to((np.asarray(bu, f32) * 0.5)[None, :], (P, K)))
    in_maps = []
    for c in range(NCORES):
        b, r = c // 2, c % 2
        xb = np.asarray(x[b], f32)
        xT = np.ascontiguousarray(xb.T)
        heads = range(r * HL, r * HL + HL)
        wm = np.concatenate([M[h] for h in heads], axis=1)
        wg = np.concatenate([G[h] for h in heads], axis=1)
        in_maps.append({
            "xT": xT.astype(bf16),
            "x8": np.clip(xT * SX, -240.0, 240.0).astype(fp8),
            "wm": wm.astype(bf16),
            "wg": wg.astype(bf16),
            "bias_bc": bias_bc,
        })
    return in_maps


def _assemble(results):
    out = np.empty((B, T, K), np.float32)
    for c in range(NCORES):
        b, r = c // 2, c % 2
        o = np.asarray(results[c]["out"], np.float32)
        for qb in range(QB):  # 4 chunks of 256 tokens
            t0 = qb * 512 + r * 256
            out[b, t0:t0 + 256] = o[qb * 256:(qb + 1) * 256]
    return out


def run_on_hw(x, Wq, Wk, Wv, Wu, bu, trace=False, tmpdir=None):
    from concourse.bass_utils import run_bass_kernel_spmd
    nc = _get_nc()
    in_maps = _make_in_maps(x, Wq, Wk, Wv, Wu, bu)
    res = run_bass_kernel_spmd(nc, in_maps, core_ids=list(range(NCORES)),
                               trace=trace, tmpdir=tmpdir)
    return _assemble(res.results), res


def kernel(x, Wq, Wk, Wv, Wu, bu):
    out, _ = run_on_hw(x, Wq, Wk, Wv, Wu, bu, trace=False)
    return out

